# revision 14
# baseline (speedup 1.0000x reference)
"""MoE MLP (shared expert weights => plain two-layer GELU MLP) on 8 trn2 cores.

Math (routing is an identity permutation, so gating is dead code):
    h   = gelu(x @ proj1.T + b1)        x: [L, N, E] -> tokens [T=L*N, E]
    out = h @ proj2.T + b2              out: [T, E] -> [L, N, E]

Sharding: data parallel over the token dim (T=16384 -> 2048 tokens/core),
weights replicated.

Per core the kernel is FUSED over 4 token blocks of 512: for each block,
pass 1 (hT = gelu(w1T.T @ xT + b1)) writes the hidden activation straight
into an 8MB SBUF tile (mxn_subtile_producer), and pass 2
(outT = w2T.T @ hT + b2) consumes that tile as the matmul moving operand
with ZERO hT DMA. Versus the unfused 2-pass version this removes the
160MB/core hT DRAM round trip, the pass1->pass2 PE gap, and the 8MB
x-preload stall region at kernel start (x per block is only 2MB).

Weights stream per block (w1+w2 = 64MB/block, ~145GB/s per phase, well
under the ~355GB/s core DMA roofline). Explicit prefetch of the first
strips of each phase during the previous phase keeps every phase boundary
off the critical path. The pass-2 reducer DMAs each output subtile in two
128KB halves directly, so the post-matmul tail is one small write, not a
1MB super-tile flush.

All matmul operands are bf16 (host-cast); PSUM accumulation and the
epilogue (exact-erf GELU + biases on ScalarE) stay fp32. A short burst of
dummy matmuls on a zeroed SBUF tile overlaps the initial DMAs so the PE
HAM clock-gate is already at 8/8 when real work arrives.
"""

import numpy as np

_L, _N, _E, _H = 2048, 8, 2048, 8192
_T = _L * _N            # 16384 tokens
_NCORES = 8
_TS = _T // _NCORES     # 2048 tokens per core
_P = 128
_NB = 4                 # token blocks per core
_BS = _TS // _NB        # 512 tokens per block

_WARMUP_MMS = 14        # dummy matmuls overlapping the head DMAs
_W2PRE = 4              # pass-2 weight strips prefetched during pass 1

_compiled_nc = None


def _build_nc():
    from contextlib import ExitStack

    import concourse.bacc as bacc
    import concourse.mybir as mybir
    import concourse.tile as tile
    from concourse.kernels.tile_matmul import (
        ShapeInfo,
        composable_matmul_tile_kernel,
    )

    f32 = mybir.dt.float32
    bf16 = mybir.dt.bfloat16

    nc = bacc.Bacc(None, target_bir_lowering=False, debug=False)
    with tile.TileContext(nc) as tc:
        with ExitStack() as ctx:
            dram = ctx.enter_context(tc.tile_pool(name="dram", bufs=1, space="DRAM"))
            # host-pre-tiled layouts (see _make_in_maps for index math)
            xt_t = dram.tile([_NB, 8, _P, 1024], bf16, kind="ExternalInput", name="xt_t", uniquify=False)
            w1t = dram.tile([16, 8, _P, 1024], bf16, kind="ExternalInput", name="w1t", uniquify=False)
            w2t = dram.tile([4, 16, _P, 2048], bf16, kind="ExternalInput", name="w2t", uniquify=False)
            b1r = dram.tile([_P, _H // _P], f32, kind="ExternalInput", name="b1r", uniquify=False)
            b2r = dram.tile([_P, _E // _P], f32, kind="ExternalInput", name="b2r", uniquify=False)
            outT = dram.tile([_E, _TS], f32, kind="ExternalOutput", name="outT", uniquify=False)
            outT3 = outT[:].rearrange("(po pi) f -> pi po f", pi=_P)

            const = ctx.enter_context(tc.tile_pool(name="const", bufs=1))
            b1_sb = const.tile([_P, _H // _P], f32, name="b1_sb")
            nc.sync.dma_start(b1_sb[:], b1r[:])
            b2_sb = const.tile([_P, _E // _P], f32, name="b2_sb")
            nc.sync.dma_start(b2_sb[:], b2r[:])

            # --- persistent pools ---
            # prefetch pools hold 2 blocks' worth of tiles so a prefetch DMA
            # never WAR-waits on the previous block's readers (a parked DMA
            # would block its whole queue FIFO behind it)
            xpre = ctx.enter_context(tc.tile_pool(name="xpre", bufs=16))
            w1pre = ctx.enter_context(tc.tile_pool(name="w1pre", bufs=16))
            p1_kxm_pool = ctx.enter_context(tc.tile_pool(name="p1_kxm", bufs=6))
            w2pre = ctx.enter_context(tc.tile_pool(name="w2pre", bufs=2 * _W2PRE))
            p2_kxm_pool = ctx.enter_context(tc.tile_pool(name="p2_kxm", bufs=6))
            # hT lives entirely in SBUF as 16 per-strip tiles (pass1 m-tile ==
            # pass2 k-tile granularity). Separate tiles keep the dependency
            # tracking per-strip: pass2's first matmul must not wait on
            # pass1's LAST gelu, only on the strip it actually reads.
            ht_pool = ctx.enter_context(tc.tile_pool(name="ht", bufs=1))
            hT_strips = [
                ht_pool.tile([_P, 4, 512], bf16, name=f"hTs{kt}") for kt in range(16)
            ]

            xtiles = {}   # (nb, kt) -> x chunk [128, 2, 512]
            w1tiles = {}  # (nb, kt) -> w1 strip for m_tile 0
            w2tiles = {}  # (nb, kt) -> w2 strip for m_tile 0, kt < _W2PRE

            def prefetch_block(nb):
                # next block's x chunks + first-m-row w1 strips; issued in
                # program order before the previous phase so the DMAs land
                # while the PE is busy there
                for kt in range(8):
                    t = xpre.tile([_P, 2, _BS], bf16, name=f"xc{nb}_{kt}", tag="xc")
                    nc.sync.dma_start(
                        t[:],
                        xt_t[:][nb, kt].rearrange("pi (ks f) -> pi ks f", ks=2),
                    )
                    xtiles[(nb, kt)] = t
                    w = w1pre.tile([_P, 2, 512], bf16, name=f"w1p{nb}_{kt}", tag="w1p")
                    nc.sync.dma_start(
                        w[:],
                        w1t[:][0, kt].rearrange("pi (ks f) -> pi ks f", ks=2),
                    )
                    w1tiles[(nb, kt)] = w

            def prefetch_w2(nb, count=_W2PRE):
                for kt in range(count):
                    w = w2pre.tile([_P, 4, 512], bf16, name=f"w2p{nb}_{kt}", tag="w2p")
                    nc.sync.dma_start(
                        w[:],
                        w2t[:][0, kt].rearrange("pi (ks f) -> pi ks f", ks=4),
                    )
                    w2tiles[(nb, kt)] = w

            def gelu_reducer(nc_, psum, sbuf, md):
                # global 128-row group of H for this psum subtile
                g = md.m_tile_idx * md.m_subtiles + md.m_subtile_idx
                nc_.scalar.activation(
                    sbuf,
                    psum,
                    mybir.ActivationFunctionType.Gelu,
                    bias=b1_sb[:, g : g + 1],
                )

            # issue the block-0 prefetches first so they head the DMA queues.
            # block 0's w2 prefetch is trimmed to one strip: the head is the
            # only DMA-bandwidth-tight window (x + w1 + first JIT strips all
            # land in the first ~28us), and pass 2 is 221us away
            prefetch_block(0)
            prefetch_w2(0, count=1)

            # --- PE warmup: dummy matmuls on a zeroed tile overlap the head
            # DMAs so HAM reaches 8/8 before the first real matmul ---
            if _WARMUP_MMS:
                warm = const.tile([_P, 640], bf16, name="warm")
                nc.any.memset(warm[:], 0)
                # transient psum pool: releases its bank before the real work
                with tc.tile_pool(name="wpsum", bufs=1, space="PSUM") as wps:
                    wp = wps.tile([_P, 512], f32, name="wp")
                    for _ in range(_WARMUP_MMS):
                        nc.tensor.matmul(
                            wp[:], warm[:, :128], warm[:, 128:640], start=True, stop=True
                        )

            for nb in range(_NB):
                # ---- pass 1: hT_strips = gelu(w1T.T @ xT_blk + b1) ----
                def p1_kxm_producer(nc_, md, nb=nb):
                    t = w1tiles.pop((nb, md.k_tile_idx), None)
                    if t is not None and md.m_tile_idx == 0:
                        return t
                    t = p1_kxm_pool.tile([_P, 2, 512], bf16, name="p1kxm", tag="p1kxm")
                    nc_.sync.dma_start(
                        t[:],
                        w1t[:][md.m_tile_idx, md.k_tile_idx].rearrange(
                            "pi (ks f) -> pi ks f", ks=2
                        ),
                    )
                    return t

                def p1_kxn_producer(nc_, md, nb=nb):
                    return xtiles[(nb, md.k_tile_idx)]

                def hT_subtile_producer(nc_, md):
                    return hT_strips[md.m_tile_idx][:]

                composable_matmul_tile_kernel(
                    tc,
                    kxm_shape=ShapeInfo(pdims=((_P, _E // _P),), fdims=(_H,)),
                    kxn_shape=ShapeInfo(pdims=((_P, _E // _P),), fdims=(_BS,)),
                    output_type=None,
                    kxm_producer=p1_kxm_producer,
                    kxn_producer=p1_kxn_producer,
                    mxn_consumer=lambda nc_, sbuf, md: None,
                    mxn_subtile_producer=hT_subtile_producer,
                    mxn_subtile_reducer=gelu_reducer,
                    MAX_K_TILE_SIZE=256,
                    temps_n_bufs=1,
                    psum_n_bufs=2,
                )

                # next block's x + w1 loads overlap this block's pass 2
                if nb + 1 < _NB:
                    prefetch_block(nb + 1)
                    prefetch_w2(nb + 1)

                # ---- pass 2: outT_blk = w2T.T @ hTsb + b2 ----
                def p2_kxm_producer(nc_, md, nb=nb):
                    t = w2tiles.pop((nb, md.k_tile_idx), None)
                    if t is not None and md.m_tile_idx == 0:
                        return t
                    t = p2_kxm_pool.tile([_P, 4, 512], bf16, name="p2kxm", tag="p2kxm")
                    nc_.sync.dma_start(
                        t[:],
                        w2t[:][md.m_tile_idx, md.k_tile_idx].rearrange(
                            "pi (ks f) -> pi ks f", ks=4
                        ),
                    )
                    return t

                def p2_kxn_producer(nc_, md):
                    return hT_strips[md.k_tile_idx][:]

                def bias_reducer(nc_, psum, sbuf, md, nb=nb):
                    g = md.m_tile_idx * md.m_subtiles + md.m_subtile_idx
                    nc_.scalar.activation(
                        sbuf,
                        psum,
                        mybir.ActivationFunctionType.Identity,
                        bias=b2_sb[:, g : g + 1],
                    )
                    # write this subtile straight out in four quarters (the
                    # mxn_consumer is a no-op) so the kernel tail is a few
                    # 64KB DMAs on separate queues, not a 1MB super-tile flush
                    for h in range(4):
                        sl = slice(nb * _BS + h * 128, nb * _BS + (h + 1) * 128)
                        nc_.sync.dma_start(
                            outT3[:, g : g + 1, sl],
                            sbuf[:, :, h * 128 : (h + 1) * 128],
                        )

                composable_matmul_tile_kernel(
                    tc,
                    kxm_shape=ShapeInfo(pdims=((_P, _H // _P),), fdims=(_E,)),
                    kxn_shape=ShapeInfo(pdims=((_P, _H // _P),), fdims=(_BS,)),
                    output_type=f32,
                    kxm_producer=p2_kxm_producer,
                    kxn_producer=p2_kxn_producer,
                    mxn_consumer=lambda nc_, sbuf, md: None,
                    mxn_subtile_reducer=bias_reducer,
                    MAX_K_TILE_SIZE=512,
                    temps_n_bufs=1,
                    psum_n_bufs=2,
                )

    nc.compile()
    return nc


def _get_nc():
    global _compiled_nc
    if _compiled_nc is None:
        _compiled_nc = _build_nc()
    return _compiled_nc


def _make_in_maps(x, proj1, proj1_bias, proj2, proj2_bias):
    import ml_dtypes

    bf16 = ml_dtypes.bfloat16
    xt = np.ascontiguousarray(x.reshape(_T, _E))
    # per-SBUF-tile contiguous layouts (index math validated vs the naive
    # formulas): w1t[mt,kt,pi,ks*512+f] = proj1.T[kt*256+ks*128+pi, mt*512+f]
    w1t = np.ascontiguousarray(
        proj1.T.astype(bf16)
        .reshape(8, 2, 128, 16, 512)
        .transpose(3, 0, 2, 1, 4)
        .reshape(16, 8, 128, 1024)
    )
    # w2t[mt,kt,pi,ks*512+f] = proj2.T[kt*512+ks*128+pi, mt*512+f]
    w2t = np.ascontiguousarray(
        proj2.T.astype(bf16)
        .reshape(16, 4, 128, 4, 512)
        .transpose(3, 0, 2, 1, 4)
        .reshape(4, 16, 128, 2048)
    )
    b1r = np.ascontiguousarray(proj1_bias.reshape(_H // _P, _P).T)
    b2r = np.ascontiguousarray(proj2_bias.reshape(_E // _P, _P).T)
    in_maps = []
    for c in range(_NCORES):
        shard_T = xt[c * _TS : (c + 1) * _TS].T  # [E, TS]
        # xt_t[nb,kt,pi,ks*512+f] = xT[kt*256+ks*128+pi, nb*512+f]
        xt_tiled = np.ascontiguousarray(
            shard_T.astype(bf16)
            .reshape(8, 2, 128, _NB, _BS)
            .transpose(3, 0, 2, 1, 4)
            .reshape(_NB, 8, 128, 1024)
        )
        in_maps.append(
            {"xt_t": xt_tiled, "w1t": w1t, "w2t": w2t, "b1r": b1r, "b2r": b2r}
        )
    return in_maps


def kernel(x, proj1, proj1_bias, proj2, proj2_bias, gate_w=None, **_ignored):
    # gate_w only affects the (dead) routing ids, never the output.
    from concourse.bass_utils import run_bass_kernel_spmd

    nc = _get_nc()
    in_maps = _make_in_maps(
        np.asarray(x, np.float32),
        np.asarray(proj1, np.float32),
        np.asarray(proj1_bias, np.float32),
        np.asarray(proj2, np.float32),
        np.asarray(proj2_bias, np.float32),
    )
    res = run_bass_kernel_spmd(nc, in_maps, list(range(_NCORES)))
    out = np.empty((_T, _E), np.float32)
    for c in range(_NCORES):
        out[c * _TS : (c + 1) * _TS] = res.results[c]["outT"].T
    return out.reshape(_L, _N, _E)


# revision 23
# speedup vs baseline: 1.0064x; 1.0064x over previous
"""MoE MLP (shared expert weights => plain two-layer GELU MLP) on 8 trn2 cores.

Math (routing is an identity permutation, so gating is dead code):
    h   = gelu(x @ proj1.T + b1)        x: [L, N, E] -> tokens [T=L*N, E]
    out = h @ proj2.T + b2              out: [T, E] -> [L, N, E]

Sharding: data parallel over the token dim (T=16384 -> 2048 tokens/core),
weights replicated.

Per core the kernel is FUSED over 4 token blocks of 512 tokens: for each
block, pass 1 (hT = gelu(w1T.T @ xT + b1)) writes the hidden activation
straight into 16 SBUF strip tiles (8MB total), and pass 2
(outT = w2T.T @ hT + b2) consumes those strips as the matmul moving
operand with ZERO hT DMA. Versus an unfused 2-pass version this removes
the 160MB/core hT DRAM round trip, the pass1->pass2 PE gap, and the 8MB
x-preload stall at kernel start (x per block is only 2MB).

The inner matmul loop is written directly (not via
composable_matmul_tile_kernel) so the PSUM and output-staging pools are
created ONCE and persist across all 8 phases: per-phase pool
creation/teardown makes each phase's first matmul WAR-wait on the
previous phase's final reducers (pool-granularity tracking), ~1us per
boundary. With persistent pools the per-name buffer rotation (bufs=2)
never collides with the immediately preceding super-tile.

Other scheduling choices, all verified against neuron-profile traces:
  - every dma_start costs ~0.6us of serialized Sync-engine time, so the
    output is written as ONE dma_start per 128-row subtile - the last
    super-tile's sync ops run after the final matmul and set the tail;
  - the first x chunk + w1 strip are split across 4 DMA queues (a single
    256KB dma_start on one queue takes ~12us - it would gate the first
    matmul);
  - x chunks / first-row w1 strips / first w2 strips of each phase are
    prefetched one phase ahead in pools sized so a prefetch never
    WAR-parks (a parked DMA blocks its whole queue FIFO);
  - a short burst of dummy matmuls on a zeroed tile overlaps the head
    DMAs so the PE HAM clock-gate is at 8/8 when real work arrives.

All matmul operands are bf16 (host-cast); PSUM accumulation and the
epilogue (exact-erf GELU + biases on ScalarE) stay fp32.
"""

import numpy as np

_L, _N, _E, _H = 2048, 8, 2048, 8192
_T = _L * _N            # 16384 tokens
_NCORES = 8
_TS = _T // _NCORES     # 2048 tokens per core
_P = 128
_NB = 4                 # token blocks per core
_BS = _TS // _NB        # 512 tokens per block

_WARMUP_MMS = 10        # dummy matmuls overlapping the head DMAs
_W2PRE = 4              # pass-2 weight strips prefetched during pass 1

_compiled_nc = None


def _build_nc():
    from contextlib import ExitStack

    import concourse.bacc as bacc
    import concourse.mybir as mybir
    import concourse.tile as tile

    f32 = mybir.dt.float32
    bf16 = mybir.dt.bfloat16

    nc = bacc.Bacc(None, target_bir_lowering=False, debug=False)
    with tile.TileContext(nc) as tc:
        with ExitStack() as ctx:
            dram = ctx.enter_context(tc.tile_pool(name="dram", bufs=1, space="DRAM"))
            # host-pre-tiled layouts (see _make_in_maps for index math)
            xt_t = dram.tile([_NB, 8, _P, 1024], bf16, kind="ExternalInput", name="xt_t", uniquify=False)
            w1t = dram.tile([16, 8, _P, 1024], bf16, kind="ExternalInput", name="w1t", uniquify=False)
            w2t = dram.tile([4, 16, _P, 2048], bf16, kind="ExternalInput", name="w2t", uniquify=False)
            b1r = dram.tile([_P, _H // _P], f32, kind="ExternalInput", name="b1r", uniquify=False)
            b2r = dram.tile([_P, _E // _P], f32, kind="ExternalInput", name="b2r", uniquify=False)
            outT = dram.tile([_E, _TS], f32, kind="ExternalOutput", name="outT", uniquify=False)
            outT3 = outT[:].rearrange("(po pi) f -> pi po f", pi=_P)

            const = ctx.enter_context(tc.tile_pool(name="const", bufs=1))
            b1_sb = const.tile([_P, _H // _P], f32, name="b1_sb")
            nc.sync.dma_start(b1_sb[:], b1r[:])
            b2_sb = const.tile([_P, _E // _P], f32, name="b2_sb")
            nc.sync.dma_start(b2_sb[:], b2r[:])

            # --- persistent pools ---
            # prefetch pools hold 2 blocks' worth of tiles so a prefetch DMA
            # never WAR-waits on the previous block's readers
            xpre = ctx.enter_context(tc.tile_pool(name="xpre", bufs=16))
            w1pre = ctx.enter_context(tc.tile_pool(name="w1pre", bufs=16))
            p1_kxm_pool = ctx.enter_context(tc.tile_pool(name="p1_kxm", bufs=5))
            w2pre = ctx.enter_context(tc.tile_pool(name="w2pre", bufs=_W2PRE + 2))
            p2_kxm_pool = ctx.enter_context(tc.tile_pool(name="p2_kxm", bufs=5))
            # hT: 16 per-strip tiles (pass1 m-tile == pass2 k-tile). Separate
            # tiles keep dependency tracking per-strip.
            ht_pool = ctx.enter_context(tc.tile_pool(name="ht", bufs=1))
            hT_strips = [
                ht_pool.tile([_P, 4, 512], bf16, name=f"hTs{kt}") for kt in range(16)
            ]

            xtiles = {}   # (nb, kt) -> x chunk [128, 2, 512]
            w1tiles = {}  # (nb, kt) -> w1 strip for m_tile 0
            w2tiles = {}  # (nb, kt) -> w2 strip for m_tile 0, kt < _W2PRE

            def prefetch_block(nb, split_first=False):
                for kt in range(8):
                    t = xpre.tile([_P, 2, _BS], bf16, name=f"xc{nb}_{kt}", tag="xc")
                    src = xt_t[:][nb, kt].rearrange("pi (ks f) -> pi ks f", ks=2)
                    w = w1pre.tile([_P, 2, 512], bf16, name=f"w1p{nb}_{kt}", tag="w1p")
                    wsrc = w1t[:][0, kt].rearrange("pi (ks f) -> pi ks f", ks=2)
                    if split_first and kt == 0:
                        # 4 dma_starts each on separate queues: a single-queue
                        # 256KB transfer would gate the first matmul by ~12us
                        for s in range(2):
                            sl = slice(s * 256, (s + 1) * 256)
                            for j in range(2):
                                nc.sync.dma_start(t[:, j : j + 1, sl], src[:, j : j + 1, sl])
                                nc.sync.dma_start(w[:, j : j + 1, sl], wsrc[:, j : j + 1, sl])
                    else:
                        nc.sync.dma_start(t[:], src)
                        nc.sync.dma_start(w[:], wsrc)
                    xtiles[(nb, kt)] = t
                    w1tiles[(nb, kt)] = w

            def prefetch_w2(nb, count=_W2PRE):
                for kt in range(count):
                    w = w2pre.tile([_P, 4, 512], bf16, name=f"w2p{nb}_{kt}", tag="w2p")
                    nc.sync.dma_start(
                        w[:],
                        w2t[:][0, kt].rearrange("pi (ks f) -> pi ks f", ks=4),
                    )
                    w2tiles[(nb, kt)] = w

            # block-0 prefetches head the DMA queues; the first chunk/strip
            # are split 4-ways so the first matmul isn't gated by a single
            # queue moving 256KB. Block 0's w2 prefetch is trimmed: the head
            # is the only bandwidth-tight window and pass 2 is 221us away.
            prefetch_block(0, split_first=True)
            prefetch_w2(0, count=1)

            # --- PE warmup: dummy matmuls on a zeroed tile overlap the head
            # DMAs so HAM reaches 8/8 before the first real matmul ---
            if _WARMUP_MMS:
                warm = const.tile([_P, 640], bf16, name="warm")
                nc.any.memset(warm[:], 0)
                with tc.tile_pool(name="wpsum", bufs=1, space="PSUM") as wps:
                    wp = wps.tile([_P, 512], f32, name="wp")
                    for _ in range(_WARMUP_MMS):
                        nc.tensor.matmul(
                            wp[:], warm[:, :128], warm[:, 128:640], start=True, stop=True
                        )

            # ONE psum/staging pool for all phases (created after the warmup
            # pool is closed so all 8 PSUM banks are free). temps has 8 bufs:
            # 4 subtile allocs per super-tile x 2 super-tiles in flight, so a
            # reducer never WAR-waits on a still-running output DMA.
            psum = ctx.enter_context(tc.tile_pool(name="psum", bufs=2, space="PSUM"))
            temps = ctx.enter_context(tc.tile_pool(name="temps", bufs=6))

            def matmul_phase(
                n_m,          # number of 512-row output tiles
                n_k,          # number of k tiles
                k_sub,        # 128-row k subtiles per k tile
                kxm_get,      # (mt, kt) -> [128, k_sub, 512] stationary strip
                kxn_get,      # (kt) -> [128, k_sub, 512] moving strip
                reduce_sub,   # (psum_ap, mt, mi) -> None  (psum -> dest)
            ):
                for mt in range(n_m):
                    ps = [
                        psum.tile([_P, 512], f32, name=f"ps{j}") for j in range(4)
                    ]
                    for kt in range(n_k):
                        kxm = kxm_get(mt, kt)
                        kxn = kxn_get(kt)
                        for mi in range(4):
                            for ks in range(k_sub):
                                nc.tensor.matmul(
                                    ps[mi][:],
                                    kxm[:, ks, mi * _P : (mi + 1) * _P],
                                    kxn[:, ks, :],
                                    start=(kt == 0 and ks == 0),
                                    stop=(kt == n_k - 1 and ks == k_sub - 1),
                                )
                    for mi in range(4):
                        reduce_sub(ps[mi], mt, mi)

            for nb in range(_NB):
                # ---- pass 1: hT_strips = gelu(w1T.T @ xT_blk + b1) ----
                def p1_kxm(mt, kt, nb=nb):
                    t = w1tiles.pop((nb, kt), None) if mt == 0 else None
                    if t is not None:
                        return t[:]
                    t = p1_kxm_pool.tile([_P, 2, 512], bf16, name="p1kxm", tag="p1kxm")
                    nc.sync.dma_start(
                        t[:],
                        w1t[:][mt, kt].rearrange("pi (ks f) -> pi ks f", ks=2),
                    )
                    return t[:]

                def p1_kxn(kt, nb=nb):
                    return xtiles[(nb, kt)][:]

                def p1_reduce(ps, mt, mi):
                    g = mt * 4 + mi
                    nc.scalar.activation(
                        hT_strips[mt][:, mi, :],
                        ps[:],
                        mybir.ActivationFunctionType.Gelu,
                        bias=b1_sb[:, g : g + 1],
                    )

                matmul_phase(16, 8, 2, p1_kxm, p1_kxn, p1_reduce)

                # next block's x + w1 loads overlap this block's pass 2
                if nb + 1 < _NB:
                    prefetch_block(nb + 1)
                    prefetch_w2(nb + 1)

                # ---- pass 2: outT_blk = w2T.T @ hT_strips + b2 ----
                def p2_kxm(mt, kt, nb=nb):
                    t = w2tiles.pop((nb, kt), None) if mt == 0 else None
                    if t is not None:
                        return t[:]
                    t = p2_kxm_pool.tile([_P, 4, 512], bf16, name="p2kxm", tag="p2kxm")
                    nc.sync.dma_start(
                        t[:],
                        w2t[:][mt, kt].rearrange("pi (ks f) -> pi ks f", ks=4),
                    )
                    return t[:]

                def p2_kxn(kt):
                    return hT_strips[kt][:]

                def p2_reduce(ps, mt, mi, nb=nb):
                    g = mt * 4 + mi
                    out_sb = temps.tile([_P, 512], f32, name="p2out", tag="p2out")
                    nc.scalar.activation(
                        out_sb[:],
                        ps[:],
                        mybir.ActivationFunctionType.Identity,
                        bias=b2_sb[:, g : g + 1],
                    )
                    # ONE dma_start per subtile: each dma_start costs ~0.6us
                    # of serialized Sync time and the last super-tile's sync
                    # ops run after the final matmul (they set the tail)
                    nc.sync.dma_start(
                        outT3[:, g, nb * _BS : (nb + 1) * _BS], out_sb[:]
                    )

                matmul_phase(4, 16, 4, p2_kxm, p2_kxn, p2_reduce)

    nc.compile()
    return nc


def _get_nc():
    global _compiled_nc
    if _compiled_nc is None:
        _compiled_nc = _build_nc()
    return _compiled_nc


def _make_in_maps(x, proj1, proj1_bias, proj2, proj2_bias):
    import ml_dtypes

    bf16 = ml_dtypes.bfloat16
    xt = np.ascontiguousarray(x.reshape(_T, _E))
    # per-SBUF-tile contiguous layouts (index math validated vs the naive
    # formulas): w1t[mt,kt,pi,ks*512+f] = proj1.T[kt*256+ks*128+pi, mt*512+f]
    w1t = np.ascontiguousarray(
        proj1.T.astype(bf16)
        .reshape(8, 2, 128, 16, 512)
        .transpose(3, 0, 2, 1, 4)
        .reshape(16, 8, 128, 1024)
    )
    # w2t[mt,kt,pi,ks*512+f] = proj2.T[kt*512+ks*128+pi, mt*512+f]
    w2t = np.ascontiguousarray(
        proj2.T.astype(bf16)
        .reshape(16, 4, 128, 4, 512)
        .transpose(3, 0, 2, 1, 4)
        .reshape(4, 16, 128, 2048)
    )
    b1r = np.ascontiguousarray(proj1_bias.reshape(_H // _P, _P).T)
    b2r = np.ascontiguousarray(proj2_bias.reshape(_E // _P, _P).T)
    in_maps = []
    for c in range(_NCORES):
        shard_T = xt[c * _TS : (c + 1) * _TS].T  # [E, TS]
        # xt_t[nb,kt,pi,ks*512+f] = xT[kt*256+ks*128+pi, nb*512+f]
        xt_tiled = np.ascontiguousarray(
            shard_T.astype(bf16)
            .reshape(8, 2, 128, _NB, _BS)
            .transpose(3, 0, 2, 1, 4)
            .reshape(_NB, 8, 128, 1024)
        )
        in_maps.append(
            {"xt_t": xt_tiled, "w1t": w1t, "w2t": w2t, "b1r": b1r, "b2r": b2r}
        )
    return in_maps


def kernel(x, proj1, proj1_bias, proj2, proj2_bias, gate_w=None, **_ignored):
    # gate_w only affects the (dead) routing ids, never the output.
    from concourse.bass_utils import run_bass_kernel_spmd

    nc = _get_nc()
    in_maps = _make_in_maps(
        np.asarray(x, np.float32),
        np.asarray(proj1, np.float32),
        np.asarray(proj1_bias, np.float32),
        np.asarray(proj2, np.float32),
        np.asarray(proj2_bias, np.float32),
    )
    res = run_bass_kernel_spmd(nc, in_maps, list(range(_NCORES)))
    out = np.empty((_T, _E), np.float32)
    for c in range(_NCORES):
        out[c * _TS : (c + 1) * _TS] = res.results[c]["outT"].T
    return out.reshape(_L, _N, _E)
